# revision 19
# baseline (speedup 1.0000x reference)
"""Trainium2 Bass kernel for CausalGNNLayer (per-node-type Linear, MoE-style routing).

Semantics (matching the reference):
    out[n, :] = x[n, :] @ W[node_types[n]].T + b[node_types[n]]
edge_index is unused by the op.

Strategy:
- Host-side routing-aware sharding: stable-sort nodes by type, split each
  type's node list into two halves -> 8 groups (4 types x 2 cores).
- Each core receives its group's rows of x in bfloat16, pre-packed into a
  chunk-major layout (per chunk: [128 partitions, 4 k-tiles, chunk nodes],
  chunks concatenated in one 1-D dram tensor) so every DMA descriptor is a
  multi-KB contiguous run; plus that type's weight ([in, out], bf16) and
  fp32 bias.
- On-device: dense [P,512] @ [512,512] + bias -> [P,512] bf16 with fp32
  PSUM accumulation, 128-node blocks, 4 K-tile PSUM accumulation. bf16
  halves HBM traffic vs fp32 and enables the tensor engine's fast weight
  load (fp32r disables it); rel-err ~3e-3, well inside the 2e-2 gate.
- Output is staged per chunk and written with one DMA per chunk into a
  [128, nblocks, 512] chunk-major dram layout (8KB descriptors); the host
  transposes back and scatters into the full fp32 [N, 512] output.
- DMA issue is spread across engines (x-chunks on sync, weights/bias on
  scalar/vector/gpsimd, outputs on scalar) because each dma_start costs
  ~650ns of sequencer time and a single queue serializes the pipeline.

This does the minimum flops (each node touched by exactly one weight),
unlike the reference's compute-all-4-then-mask.
"""

import numpy as np
import ml_dtypes
from contextlib import ExitStack

import concourse.bass as bass
import concourse.mybir as mybir
import concourse.tile as tile
from concourse.bass_utils import run_bass_kernel_spmd

N_CORES = 8
IN_CH = 512
OUT_CH = 512
NUM_TYPES = 4
P_BLK = 128          # SBUF partition count / node-block size
KT = IN_CH // P_BLK  # 4 contraction tiles
CHUNK_BLKS = 8       # node blocks per x DMA chunk (1024 nodes)
XBUFS = 5            # x-chunk prefetch depth
PSBUFS = 8           # PSUM bank ring depth (all 8 banks)
OBUFS = 3            # output chunk staging depth

BF16 = np.dtype(ml_dtypes.bfloat16)

# Set by test harness to capture HW profile; kernel works without it.
TRACE = False
LAST_RESULTS = None

_compile_cache: dict = {}

_legal_nop_counter = [0]


def _legalize_waits(nc: bass.Bass) -> None:
    """This walrus codegen only encodes ONE sync wait per engine instruction.
    Tile's scheduler attaches several.  Split: hoist all-but-one wait of any
    multi-wait instruction into preceding same-engine NoOps (one wait each) —
    semantically identical (the engine stalls on each wait in program order)."""
    for fn in nc.m.functions:
        for blk in fn.blocks:
            insts = blk.instructions
            out = []
            changed = False
            for inst in insts:
                si = inst.sync_info
                waits = list(si.on_wait) if si is not None and si.on_wait else []
                if len(waits) > 1:
                    changed = True
                    for w in waits[:-1]:
                        _legal_nop_counter[0] += 1
                        nop = mybir.InstNoOp(
                            name=f"waitsplit-{_legal_nop_counter[0]}",
                            ins=[],
                            outs=[],
                            engine=inst.engine,
                        )
                        nop.sync_info = mybir.SyncInfo(on_wait=[w], on_update=[])
                        out.append(nop)
                    inst.sync_info = mybir.SyncInfo(
                        on_wait=[waits[-1]], on_update=list(si.on_update or [])
                    )
                out.append(inst)
            if changed:
                blk.instructions = out


def _chunk_plan(nblocks: int) -> list[tuple[int, int]]:
    """(start_block, n_blocks) per chunk: small head chunks so the PE starts
    early, small tail chunks so the final out-DMA drains fast."""
    head = []
    rem = nblocks
    for warm in (1, 2, 4, 6):
        if rem > CHUNK_BLKS + warm:
            head.append(warm)
            rem -= warm
    tail = []
    for cool in (1, 1, 3):
        if rem > CHUNK_BLKS + cool:
            tail.append(cool)
            rem -= cool
    body = []
    while rem > 0:
        nb = min(CHUNK_BLKS, rem)
        body.append(nb)
        rem -= nb
    plan = head + body + tail[::-1]
    chunks = []
    pos = 0
    for nb in plan:
        chunks.append((pos, nb))
        pos += nb
    assert pos == nblocks
    return chunks


def _build_bass(P: int) -> bass.Bass:
    """One-core program: out[P,512] = x @ w + bias (same program on all cores)."""
    nc = bass.Bass("TRN2")
    f32 = mybir.dt.float32
    bf16 = mybir.dt.bfloat16

    assert P % P_BLK == 0
    nblocks = P // P_BLK
    chunks = _chunk_plan(nblocks)

    # x: chunk-major 1-D layout; chunk ci of nb blocks occupies
    # [128, KT, nb*128] (partition-major) starting at element 512*128*pos.
    xT = nc.dram_tensor("xT", [P_BLK * KT * P], bf16, kind="ExternalInput")
    w = nc.dram_tensor("w", [IN_CH, OUT_CH], bf16, kind="ExternalInput")
    bias = nc.dram_tensor("bias", [P_BLK, OUT_CH], f32, kind="ExternalInput")
    # out: [128, nblocks, 512] chunk-major so a chunk's store is one
    # contiguous multi-KB run per partition; host transposes back.
    out = nc.dram_tensor("out", [P_BLK, nblocks, OUT_CH], bf16, kind="ExternalOutput")

    w_v = w.ap().rearrange("(k p) o -> p k o", p=P_BLK)
    out_v = out.ap()

    def x_view(pos, nb):
        L = nb * P_BLK
        base = P_BLK * KT * pos * P_BLK
        return xT.ap()[base : base + P_BLK * KT * L].rearrange(
            "(p k j) -> p k j", p=P_BLK, k=KT
        )

    with ExitStack() as ctx:
        tc = ctx.enter_context(tile.TileContext(nc))
        wp = ctx.enter_context(tc.tile_pool(name="w", bufs=1))
        bp = ctx.enter_context(tc.tile_pool(name="b", bufs=1))
        xp = ctx.enter_context(tc.tile_pool(name="x", bufs=XBUFS))
        pp = ctx.enter_context(tc.tile_pool(name="ps", bufs=PSBUFS, space="PSUM"))
        op = ctx.enter_context(tc.tile_pool(name="o", bufs=OBUFS))

        # Startup DMAs all issue from sync, serialized ~650ns apart — the DMA
        # engines fair-share among in-flight transfers, so a staggered issue
        # order makes each item complete soon after its issue.  Order: first
        # x chunk (gates the PE), weights, second chunk, bias (gates the DVE,
        # which has 8 PSUM banks of slack).
        x_tiles = {}
        pos0, nb0 = chunks[0]
        x_sb0 = xp.tile([P_BLK, KT, nb0 * P_BLK], bf16, tag="x")
        nc.sync.dma_start(x_sb0[:], x_view(pos0, nb0))
        x_tiles[0] = x_sb0

        w_sb = wp.tile([P_BLK, KT, OUT_CH], bf16)
        nc.scalar.dma_start(w_sb[:], w_v[:, :, :])
        b_sb = bp.tile([P_BLK, OUT_CH], f32)
        nc.scalar.dma_start(b_sb[:], bias.ap())

        # PE warm-up: throwaway matmuls on an uninitialized tile run while
        # the startup DMAs are in flight, ramping the PE p-state (HAM K=8/8)
        # so the first real matmuls run at full clock.  They reuse one PSUM
        # ring slot and their results are never read, so the garbage input
        # is harmless.
        gp = ctx.enter_context(tc.tile_pool(name="wm", bufs=1))
        g_sb = gp.tile([P_BLK, P_BLK + OUT_CH], bf16)
        nc.vector.memset(g_sb[:], 0.0)
        wps = pp.tile([P_BLK, OUT_CH], f32, tag="ps")
        for _ in range(8):
            nc.tensor.matmul(
                wps[:],
                lhsT=g_sb[:, :P_BLK],
                rhs=g_sb[:, P_BLK:],
                start=True,
                stop=True,
            )

        for ci, (pos, nb) in enumerate(chunks):
            if ci in x_tiles:
                x_sb = x_tiles[ci]
            else:
                x_sb = xp.tile([P_BLK, KT, nb * P_BLK], bf16, tag="x")
                nc.sync.dma_start(x_sb[:], x_view(pos, nb))
            o_sb = op.tile([P_BLK, nb, OUT_CH], bf16, tag="o")
            for bi in range(nb):
                ps = pp.tile([P_BLK, OUT_CH], f32, tag="ps")
                for k in range(KT):
                    nc.tensor.matmul(
                        ps[:],
                        lhsT=x_sb[:, k, bi * P_BLK : (bi + 1) * P_BLK],
                        rhs=w_sb[:, k, :],
                        start=(k == 0),
                        stop=(k == KT - 1),
                    )
                nc.vector.tensor_add(o_sb[:, bi, :], ps[:], b_sb[:])
            # one out-DMA per chunk from the scalar engine: keeps the sync
            # sequencer free for x-chunk issue
            nc.scalar.dma_start(out_v[:, pos : pos + nb, :], o_sb[:])
    _legalize_waits(nc)
    return nc


def _get_compiled(P: int) -> bass.Bass:
    if P not in _compile_cache:
        _compile_cache[P] = _build_bass(P)
    return _compile_cache[P]


def _pack_x(xs: np.ndarray, P: int) -> np.ndarray:
    """Pack [P, 512] bf16 rows into the chunk-major device layout."""
    nblocks = P // P_BLK
    parts = []
    for pos, nb in _chunk_plan(nblocks):
        L = nb * P_BLK
        seg = xs[pos * P_BLK : pos * P_BLK + L]        # [L, 512]
        seg = np.ascontiguousarray(seg.T)              # [512, L] = [(k p), j]
        parts.append(seg.reshape(KT, P_BLK, L).transpose(1, 0, 2).reshape(-1))
    return np.concatenate(parts)


def kernel(x, edge_index, node_types, W, b):
    global LAST_RESULTS
    x = np.asarray(x, dtype=np.float32)
    nt = np.asarray(node_types).astype(np.int64)
    W = np.asarray(W, dtype=np.float32)
    b = np.asarray(b, dtype=np.float32)
    N = x.shape[0]

    # Route nodes: stable sort by type, split each type across 2 cores.
    order = np.argsort(nt, kind="stable")
    counts = np.bincount(nt, minlength=NUM_TYPES)
    groups = []
    start = 0
    for t in range(NUM_TYPES):
        c = int(counts[t])
        idx = order[start : start + c]
        start += c
        h = (c + 1) // 2
        groups.append(idx[:h])
        groups.append(idx[h:])

    P = max(1, max(len(g) for g in groups))
    P = ((P + P_BLK - 1) // P_BLK) * P_BLK

    nc = _get_compiled(P)

    in_maps = []
    for gi, g in enumerate(groups):
        t = gi // 2
        xs = np.zeros((P, IN_CH), BF16)
        if len(g):
            xs[: len(g)] = x[g].astype(BF16)
        in_maps.append(
            {
                "xT": _pack_x(xs, P),
                "w": np.ascontiguousarray(W[t].T.astype(BF16)),
                "bias": np.ascontiguousarray(
                    np.broadcast_to(b[t][None, :], (P_BLK, OUT_CH))
                ),
            }
        )

    res = run_bass_kernel_spmd(nc, in_maps, list(range(N_CORES)), trace=TRACE)
    LAST_RESULTS = res

    out = np.empty((N, OUT_CH), np.float32)
    for gi, g in enumerate(groups):
        if len(g):
            # device out is [128, nblocks, 512] chunk-major
            o = np.asarray(res.results[gi]["out"])
            o = o.transpose(1, 0, 2).reshape(P, OUT_CH)
            out[g] = o[: len(g)].astype(np.float32)
    return out
